# revision 2
# baseline (speedup 1.0000x reference)
"""Gemma4 audio block-local attention on 8 trn2 cores.

Device (bass, fp32r matmuls): q/k/v projections and the post projection —
sharded data-parallel over (batch, seq): 8 shards of 3000 tokens (+12-token
halo). Host: the tiny block-local softmax (0.4% of FLOPs) between the two
device programs.
"""
import numpy as np

import concourse.bass as bass
import concourse.mybir as mybir
import concourse.tile as tile
from concourse import bacc
from concourse.bass_utils import run_bass_kernel_spmd

f32 = mybir.dt.float32
f32r = mybir.dt.float32r

H, D, HID = 12, 128, 1536
CHUNK, PAST, CTX = 12, 12, 24
B, S = 4, 6000
SH = 3000          # tokens per core
SHH = SH + PAST    # with halo columns
N_CORES = 8

_cache = {}


def _build_gemm(n_tok, n_weights, chunks):
    """outs[w][1536, n_tok] = W_w.T-contracted projection of x [1536, n_tok]."""
    nc = bacc.Bacc(None, target_bir_lowering=False)
    xt_d = nc.dram_tensor("xt", [12, 128, n_tok], f32r, kind="ExternalInput")
    w_ds = [
        nc.dram_tensor(f"w{i}", [12, 128, HID], f32r, kind="ExternalInput")
        for i in range(n_weights)
    ]
    o_ds = [
        nc.dram_tensor(f"o{i}", [12, 128, n_tok], f32, kind="ExternalOutput")
        for i in range(n_weights)
    ]
    with tile.TileContext(nc) as tc:
        with (
            tc.tile_pool(name="wp", bufs=1) as wp,
            tc.tile_pool(name="xp", bufs=2) as xp,
            tc.tile_pool(name="ps", bufs=8, space="PSUM") as ps,
            tc.tile_pool(name="st", bufs=2) as st,
        ):
            for wi in range(n_weights):
                w_sb = wp.tile([128, 12, HID], f32r, tag="w")
                nc.sync.dma_start(out=w_sb, in_=w_ds[wi].transpose([1, 0, 2]))
                for c0, c1 in chunks:
                    cw = c1 - c0
                    x_sb = xp.tile([128, 12, cw], f32r, tag="x")
                    nc.sync.dma_start(
                        out=x_sb, in_=xt_d[:, :, c0:c1].transpose([1, 0, 2])
                    )
                    stage = st.tile([128, 12, cw], f32, tag="s")
                    for mo in range(12):
                        acc = ps.tile([128, cw], f32)
                        for ko in range(12):
                            nc.tensor.matmul(
                                acc,
                                lhsT=w_sb[:, ko, bass.ts(mo, 128)],
                                rhs=x_sb[:, ko, :],
                                start=(ko == 0),
                                stop=(ko == 11),
                            )
                        nc.vector.tensor_copy(stage[:, mo, :], acc)
                    nc.sync.dma_start(
                        out=o_ds[wi][:, :, c0:c1].transpose([1, 0, 2]), in_=stage
                    )
    nc.compile()
    return nc


def _get_programs():
    if "p1" not in _cache:
        ch1 = [(i * 502, (i + 1) * 502) for i in range(6)]        # 3012
        ch2 = [(i * 500, (i + 1) * 500) for i in range(6)]        # 3000
        _cache["p1"] = _build_gemm(SHH, 3, ch1)
        _cache["p2"] = _build_gemm(SH, 1, ch2)
    return _cache["p1"], _cache["p2"]


def _split12(w):
    # [1536, 1536] row-major -> [12, 128, 1536] (ko, p, o) of W (already W.T)
    return np.ascontiguousarray(w.reshape(12, 128, HID))


def kernel(x, pos_emb, Wq, Wk, Wv, Wrel, Wpost, per_dim_scale):
    x = np.asarray(x, np.float32)
    qscale = (D ** -0.5 / np.log(2.0)) * np.log1p(
        np.exp(np.asarray(per_dim_scale, np.float64))
    ).astype(np.float32)
    kscale = np.float32(np.log(1.0 + np.e) / np.log(2.0))
    # fold scales into the weights (rows of W scale outputs)
    Wq_s = (np.asarray(Wq, np.float32) * np.tile(qscale, H)[:, None])
    Wk_s = np.asarray(Wk, np.float32) * kscale
    Wv_s = np.asarray(Wv, np.float32)
    wq_t = _split12(np.ascontiguousarray(Wq_s.T))
    wk_t = _split12(np.ascontiguousarray(Wk_s.T))
    wv_t = _split12(np.ascontiguousarray(Wv_s.T))
    wp_t = _split12(np.ascontiguousarray(np.asarray(Wpost, np.float32).T))

    p1, p2 = _get_programs()

    # shard: core i -> batch i//2, tokens (i%2)*3000..+3000, with 12-tok halo
    in_maps = []
    for i in range(N_CORES):
        b, half = i // 2, i % 2
        t0 = half * SH
        xs = np.zeros((SHH, HID), np.float32)
        xs[PAST:] = x[b, t0 : t0 + SH]
        if half == 1:
            xs[:PAST] = x[b, t0 - PAST : t0]
        xt = np.ascontiguousarray(xs.T).reshape(12, 128, SHH)
        in_maps.append(dict(xt=xt, w0=wq_t, w1=wk_t, w2=wv_t))

    res1 = run_bass_kernel_spmd(p1, in_maps, list(range(N_CORES))).results

    # host: block-local attention (numpy), per batch
    rel_k = (np.asarray(pos_emb, np.float32) @ np.asarray(Wrel, np.float32).T)
    rel_k = rel_k.reshape(CTX, H, D)
    NB = S // CHUNK
    idx = (np.arange(NB) * CHUNK)[:, None] + np.arange(CTX)[None, :]
    qi_ = (np.arange(NB) * CHUNK)[:, None, None] + np.arange(CHUNK)[None, :, None]
    kj = (np.arange(NB) * CHUNK)[:, None, None] + np.arange(CTX)[None, None, :] - PAST
    dist = qi_ - kj
    mask = (dist >= 0) & (dist < PAST) & (kj >= 0) & (kj < S)

    attn_maps = []
    for b in range(B):
        def asm(key):
            lo = res1[2 * b][key][:, :, PAST:].reshape(HID, SH)
            hi = res1[2 * b + 1][key][:, :, PAST:].reshape(HID, SH)
            return np.concatenate([lo, hi], axis=1).T.reshape(S, H, D)

        q, k, v = asm("o0"), asm("o1"), asm("o2")
        kh = np.concatenate([np.zeros((PAST, H, D), np.float32), k], 0)
        vh = np.concatenate([np.zeros((PAST, H, D), np.float32), v], 0)
        k_ctx, v_ctx = kh[idx], vh[idx]
        qb = q.reshape(NB, CHUNK, H, D)
        ac = np.einsum("nqhd,nchd->hnqc", qb, k_ctx, optimize=True)
        bd_raw = np.einsum("nqhd,chd->hnqc", qb, rel_k, optimize=True)
        pad = np.concatenate(
            [bd_raw, np.zeros((H, NB, CHUNK, 1), np.float32)], -1
        )
        bd = pad.reshape(H, NB, CHUNK * (CTX + 1))[..., : CHUNK * CTX].reshape(
            H, NB, CHUNK, CTX
        )
        logits = np.tanh((ac + bd) / 50.0) * 50.0
        logits = np.where(mask[None], logits, -1e9)
        p = np.exp(logits - logits.max(-1, keepdims=True))
        p /= p.sum(-1, keepdims=True)
        o = np.einsum("hnqc,nchd->nqhd", p, v_ctx, optimize=True)
        ao = o.reshape(S, H * D).T  # [1536, 6000]
        for half in range(2):
            sl = np.ascontiguousarray(ao[:, half * SH : (half + 1) * SH])
            attn_maps.append(
                dict(xt=sl.reshape(12, 128, SH), w0=wp_t)
            )

    res2 = run_bass_kernel_spmd(p2, attn_maps, list(range(N_CORES))).results

    out = np.empty((B, S, HID), np.float32)
    for b in range(B):
        lo = res2[2 * b]["o0"].reshape(HID, SH)
        hi = res2[2 * b + 1]["o0"].reshape(HID, SH)
        out[b] = np.concatenate([lo, hi], axis=1).T
    return out
